# revision 28
# baseline (speedup 1.0000x reference)
"""IntLUTConv (1x1 conv as per-pixel GEMM) on 8 TRN2 NeuronCores.

Sharding: data-parallel over batch (B=8 -> one batch item per core), no
collectives. The per-core problem is DMA-roofline-bound: 16.78 MB fp32 in +
4.19 MB int8 out per rep measures 48.2 us = 435 GB/s, exactly the per-core
SBUF AXI fabric ceiling. Every engine is kept under that DMA shadow:

  x_b [Cin=256, 16384px] fp32
  -> ONE fused custom DVE op per tile: clip(-8,7) + trunc-toward-zero + cast
     fp8e4 (7 uops: min, max, is_lt, +/-0.5-eps shift, magic-constant RNE).
     Trunc is exact except a ~3e-5-wide window below each integer boundary
     (~900 of 33.5M elements flip by 1; end-to-end rel err ~4e-3, well under
     the 2e-2 gate). Halves DVE busy time vs separate clip+trunc passes.
  -> W^T @ xq on TensorE in fp8 DoubleRow mode (contraction 256 in a single
     instruction, 2 rows/cycle; fp32 PSUM, exact integer math)
  -> ACT Copy(scale=scale/64, bias=offset) PSUM->SBUF int8 (hardware cast is
     round-half-even + saturate, matching clip(round(y*scale/64+offset)))
  -> whole-image int8 staging in SBUF (double-buffered 32 KB/partition),
     drained once per image by two 2 MB DMAs on the scalar-engine HWDGE ring.

Output-path findings (HW-measured, reps-129 interleaved-pair differencing):
per-chunk gpsimd/SWDGE output DMAs cost ~7 us/rep beyond their byte share —
SWDGE Q7 descriptor rings live in SBUF partitions 0-31 and their fetch
traffic plus frequent HBM read/write interleave disrupt the input stream.
Per-chunk scalar HWDGE outs recover ~5 us; batching the output into two big
end-of-image HWDGE DMAs recovers ~7 us (43.3 us/rep vs 50.6, where 40.6 is
the pure byte-share floor at the ~516 GB/s input streaming rate).
"""
import re
import numpy as np

import concourse.bacc as bacc
import concourse.tile as tile
import concourse.mybir as mybir
from concourse.bass_utils import run_bass_kernel_spmd
from concourse.dve_spec import Spec, Src0, Zero, C0, C1, C2, C3, Bin, AluOp, minn, maxx
from concourse.dve_spec import _spill_c3_to_src1
from concourse.dve_ops import OPS, DveOp

B, CIN, COUT, H, W = 8, 256, 256, 128, 128
NPX = H * W            # 16384 pixels per batch item
FC = 2048              # pixel chunk per pipeline stage (8 chunks)
MAGIC = 12582912.0     # 1.5 * 2**23: float add forces RNE to integer grid
HALF = 0.49997         # 0.5 - eps: trunc = RNE(t - 0.5+eps) sign-adjusted

TRACE = False
_LAST_RESULTS = [None]


def _ctq_ref(in0, in1, s0, s1, imm2):
    c2 = np.maximum(np.minimum(in0, np.float32(s0)), np.float32(s1))
    neg = (c2 < 0).astype(np.float32)
    t3 = (c2 + neg).astype(np.float32) - np.float32(imm2)
    m = np.float32(MAGIC)
    return ((t3.astype(np.float32) + m).astype(np.float32) - m).astype(np.float32)


def _register_cliptrunc():
    for existing in OPS:
        if existing.name == "CLIPTRUNCQ_ANT":
            return existing
    t = Src0
    c2 = maxx(minn(t, C0), C1)            # clip to [C1, C0] = [-8.25, 7]
    neg = Bin(AluOp.IS_LT, c2, Zero)      # 1.0 iff negative
    h = neg - C2                          # -0.49997 (pos) / +0.50003 (neg)
    t3 = c2 + h
    r = (t3 + C3) - C3                    # C3 = MAGIC via in1 spill: RNE(t3)
    op = DveOp("CLIPTRUNCQ_ANT",
               Spec(body=_spill_c3_to_src1(r), reference=_ctq_ref),
               subdim=False, uops_sha={})
    OPS.append(op)
    import concourse.dve_ops as dve_ops_mod
    dve_ops_mod.CUSTOM_DVE_SPECS[op.name] = op.spec
    dve_ops_mod._SUB_OPCODE_FOR_NAME[op.name] = (
        dve_ops_mod._CUSTOM_DVE_ROW_BASE + len(OPS) - 1)
    assert dve_ops_mod._SUB_OPCODE_FOR_NAME[op.name] < 0x20
    try:
        op.compile("v3")
    except ValueError as e:
        m = re.search(r'uops_sha\["v3"\]="([0-9a-f]+)"', str(e))
        if not m:
            raise
        op.uops_sha["v3"] = m.group(1)
        op.compile("v3")
    return op


def _chunks(FC: int, tail_split: int):
    """Chunk (start, size, is_tail) list: big FC chunks, the last split into
    smaller pieces to shrink the end-of-kernel pipeline latency."""
    sizes = [(FC, False)] * (NPX // FC)
    if tail_split == 2:
        sizes = sizes[:-1] + [(FC // 2, True)] * 2
    elif tail_split == 3:
        sizes = sizes[:-1] + [(FC // 2, True)] + [(FC // 4, True)] * 2
    elif tail_split == 4:
        sizes = sizes[:-1] + [(FC // 4, True)] * 4
    out, pos = [], 0
    for s, tail in sizes:
        out.append((pos, s, tail))
        pos += s
    assert pos == NPX
    return out


def _build(scale_val: float, offset_val: float, reps: int = 1,
           FC: int = FC, work_bufs: int = 3, xq_bufs: int = 3,
           out_bufs: int = 3, ps_bufs: int = 4, out_dma: str = "scalar",
           in_dma: str = "sync", double_row: bool = True,
           tail_split: int = 2, ps_width: int = 1024,
           skip_out_dma: bool = False, out_phase: bool = True):
    op = _register_cliptrunc()
    nc = bacc.Bacc("TRN2", target_bir_lowering=False)
    x = nc.dram_tensor("x", [CIN, NPX], mybir.dt.float32, kind="ExternalInput")
    wt = nc.dram_tensor("wt", [128, 2, COUT], mybir.dt.float8e4,
                        kind="ExternalInput")
    out = nc.dram_tensor("out", [COUT, NPX], mybir.dt.int8,
                         kind="ExternalOutput")
    chunk_list = _chunks(FC, tail_split)

    with tile.TileContext(nc) as tc, \
         tc.tile_pool(name="singles", bufs=1) as singles, \
         tc.tile_pool(name="work", bufs=work_bufs) as work, \
         tc.tile_pool(name="xqp", bufs=xq_bufs) as xqp, \
         tc.tile_pool(name="outs", bufs=out_bufs) as outs, \
         tc.tile_pool(name="obig", bufs=2) as obigp, \
         tc.tile_pool(name="psum", bufs=ps_bufs, space="PSUM") as pspool:
        wtd = singles.tile([128, 2, COUT], mybir.dt.float8e4, tag="wtd")
        nc.scalar.dma_start(out=wtd[:, :, :], in_=wt[:, :, :])
        mg = singles.tile([128, 1], mybir.dt.float32, tag="mg")
        nc.vector.memset(mg[:, :], MAGIC)

        for rep in range(reps):
          obig = (obigp.tile([128, 2, NPX], mybir.dt.int8, tag="obig",
                             name="obig")
                  if out_phase else None)
          for c0, fc, is_tail in chunk_list:
            cols = slice(c0, c0 + fc)
            xq = xqp.tile([128, 2, FC], mybir.dt.float8e4, tag="xq")
            if in_dma == "fuse" and not is_tail:
                xr2 = work.tile([128, 2, FC], mybir.dt.float32, tag="xr2")
                nc.sync.dma_start(
                    out=xr2[:, :, :fc],
                    in_=x[:, cols].rearrange("(two p) n -> p two n", two=2))
                for ct in range(2):
                    nc.vector._custom_dve(op, out=xq[:, ct, :fc],
                                          in0=xr2[:, ct, :fc],
                                          in1=mg[:, :], s0=7.0, s1=-8.25,
                                          imm2=HALF)
            else:
                for ct in range(2):
                    xr = work.tile([128, FC], mybir.dt.float32, tag=f"xr{ct}")
                    eng_in = {"alt": ("sync", "scalar"),
                              "altpool": ("sync", "gpsimd"),
                              "fuse": ("sync", "sync")}.get(
                                  in_dma, (in_dma, in_dma))[ct]
                    getattr(nc, eng_in).dma_start(
                        out=xr[:, :fc], in_=x[ct * 128:(ct + 1) * 128, cols])
                    nc.vector._custom_dve(op, out=xq[:, ct, :fc],
                                          in0=xr[:, :fc],
                                          in1=mg[:, :], s0=7.0, s1=-8.25,
                                          imm2=HALF)
            for o in range(2):
                oi8 = (obig[:, o, :] if out_phase else
                       outs.tile([128, FC], mybir.dt.int8, tag=f"oi8{o}"))
                ob = c0 if out_phase else 0
                pw = min(fc, ps_width)
                for q in range(fc // pw):
                    ps = pspool.tile([128, ps_width], mybir.dt.float32,
                                     tag="ps")
                    for k in range(pw // 256):
                        nb = q * (pw // 256) + k
                        if double_row:
                            nc.tensor.matmul(
                                ps[:, k * 256:(k + 1) * 256],
                                wtd[:, :, o * 128:(o + 1) * 128],
                                xq[:, :, nb * 256:(nb + 1) * 256],
                                start=True, stop=True,
                                perf_mode=mybir.MatmulPerfMode.DoubleRow,
                            )
                        else:
                            for ct in range(2):
                                nc.tensor.matmul(
                                    ps[:, k * 256:(k + 1) * 256],
                                    wtd[:, ct, o * 128:(o + 1) * 128],
                                    xq[:, ct, nb * 256:(nb + 1) * 256],
                                    start=(ct == 0), stop=(ct == 1),
                                )
                    nc.scalar.activation(
                        out=oi8[:, ob + q * pw:ob + (q + 1) * pw],
                        in_=ps[:, :pw],
                        func=mybir.ActivationFunctionType.Copy,
                        scale=scale_val / 64.0, bias=offset_val,
                    )
                if out_phase:
                    continue  # drained in the per-rep output phase below
                if skip_out_dma and not (is_tail and o == 1 and
                                         c0 + fc == NPX):
                    continue  # timing probe only: drop output traffic
                eng = "scalar" if is_tail else out_dma
                getattr(nc, eng).dma_start(
                    out=out[o * 128:(o + 1) * 128, cols], in_=oi8[:, :fc])
          if out_phase:
            for o in range(2):
                getattr(nc, (out_dma, "scalar")[o]).dma_start(
                    out=out[o * 128:(o + 1) * 128, :], in_=obig[:, o, :])
    nc.finalize()
    return nc


_KERNEL_CACHE: dict = {}


def _prep_wt(weights: np.ndarray) -> np.ndarray:
    """Host-side DoubleRow weight layout: wt[p, i, oc] = W[oc, p + 128*i]."""
    dt_f8 = mybir.dt.np(mybir.dt.float8e4)
    wt = weights.astype(np.float32).T            # [Cin, Cout]
    wtd = wt.reshape(2, 128, COUT).transpose(1, 0, 2)
    return np.ascontiguousarray(wtd).astype(dt_f8)


def _prep_in_maps(x: np.ndarray, weights: np.ndarray) -> list:
    wtd = _prep_wt(weights)
    return [
        {"x": np.ascontiguousarray(x[b].reshape(CIN, NPX)), "wt": wtd}
        for b in range(B)
    ]


def kernel(x, weights, scale, offset):
    x = np.asarray(x)
    weights = np.asarray(weights)
    sv = float(np.asarray(scale))
    ov = float(np.asarray(offset))

    key = (sv, ov)
    if key not in _KERNEL_CACHE:
        _KERNEL_CACHE[key] = _build(sv, ov)
    nc = _KERNEL_CACHE[key]

    in_maps = _prep_in_maps(x, weights)
    res = run_bass_kernel_spmd(nc, in_maps, core_ids=list(range(B)),
                               trace=TRACE)
    _LAST_RESULTS[0] = res
    return np.stack([r["out"].reshape(COUT, H, W) for r in res.results])


# revision 29
# speedup vs baseline: 1.0283x; 1.0283x over previous
"""IntLUTConv (1x1 conv as per-pixel GEMM) on 8 TRN2 NeuronCores.

Sharding: data-parallel over batch (B=8 -> one batch item per core), no
collectives. The per-core problem is DMA-roofline-bound: 16.78 MB fp32 in +
4.19 MB int8 out per rep measures 48.2 us = 435 GB/s, exactly the per-core
SBUF AXI fabric ceiling. Every engine is kept under that DMA shadow:

  x_b [Cin=256, 16384px] fp32
  -> ONE fused custom DVE op per tile: clip(-8,7) + trunc-toward-zero + cast
     fp8e4 (7 uops: min, max, is_lt, +/-0.5-eps shift, magic-constant RNE).
     Trunc is exact except a ~3e-5-wide window below each integer boundary
     (~900 of 33.5M elements flip by 1; end-to-end rel err ~4e-3, well under
     the 2e-2 gate). Halves DVE busy time vs separate clip+trunc passes.
  -> W^T @ xq on TensorE in fp8 DoubleRow mode (contraction 256 in a single
     instruction, 2 rows/cycle; fp32 PSUM, exact integer math)
  -> ACT Copy(scale=scale/64, bias=offset) PSUM->SBUF int8 (hardware cast is
     round-half-even + saturate, matching clip(round(y*scale/64+offset)))
  -> whole-image int8 staging in SBUF (double-buffered 32 KB/partition),
     drained once per image by two 2 MB DMAs on the scalar-engine HWDGE ring.

Output-path findings (HW-measured, reps-129 interleaved-pair differencing):
per-chunk gpsimd/SWDGE output DMAs cost ~7 us/rep beyond their byte share —
SWDGE Q7 descriptor rings live in SBUF partitions 0-31 and their fetch
traffic plus frequent HBM read/write interleave disrupt the input stream.
Per-chunk scalar HWDGE outs recover ~5 us; batching the output into two big
end-of-image HWDGE DMAs recovers ~7 us (43.3 us/rep vs 50.6, where 40.6 is
the pure byte-share floor at the ~516 GB/s input streaming rate).
"""
import re
import numpy as np

import concourse.bacc as bacc
import concourse.tile as tile
import concourse.mybir as mybir
from concourse.bass_utils import run_bass_kernel_spmd
from concourse.dve_spec import Spec, Src0, Zero, C0, C1, C2, C3, Bin, AluOp, minn, maxx
from concourse.dve_spec import _spill_c3_to_src1
from concourse.dve_ops import OPS, DveOp

B, CIN, COUT, H, W = 8, 256, 256, 128, 128
NPX = H * W            # 16384 pixels per batch item
FC = 2048              # pixel chunk per pipeline stage (8 chunks)
MAGIC = 12582912.0     # 1.5 * 2**23: float add forces RNE to integer grid
HALF = 0.49997         # 0.5 - eps: trunc = RNE(t - 0.5+eps) sign-adjusted

TRACE = False
_LAST_RESULTS = [None]


def _ctq_ref(in0, in1, s0, s1, imm2):
    c2 = np.maximum(np.minimum(in0, np.float32(s0)), np.float32(s1))
    neg = (c2 < 0).astype(np.float32)
    t3 = (c2 + neg).astype(np.float32) - np.float32(imm2)
    m = np.float32(MAGIC)
    return ((t3.astype(np.float32) + m).astype(np.float32) - m).astype(np.float32)


def _register_cliptrunc():
    for existing in OPS:
        if existing.name == "CLIPTRUNCQ_ANT":
            return existing
    t = Src0
    c2 = maxx(minn(t, C0), C1)            # clip to [C1, C0] = [-8.25, 7]
    neg = Bin(AluOp.IS_LT, c2, Zero)      # 1.0 iff negative
    h = neg - C2                          # -0.49997 (pos) / +0.50003 (neg)
    t3 = c2 + h
    r = (t3 + C3) - C3                    # C3 = MAGIC via in1 spill: RNE(t3)
    op = DveOp("CLIPTRUNCQ_ANT",
               Spec(body=_spill_c3_to_src1(r), reference=_ctq_ref),
               subdim=False, uops_sha={})
    OPS.append(op)
    import concourse.dve_ops as dve_ops_mod
    dve_ops_mod.CUSTOM_DVE_SPECS[op.name] = op.spec
    dve_ops_mod._SUB_OPCODE_FOR_NAME[op.name] = (
        dve_ops_mod._CUSTOM_DVE_ROW_BASE + len(OPS) - 1)
    assert dve_ops_mod._SUB_OPCODE_FOR_NAME[op.name] < 0x20
    try:
        op.compile("v3")
    except ValueError as e:
        m = re.search(r'uops_sha\["v3"\]="([0-9a-f]+)"', str(e))
        if not m:
            raise
        op.uops_sha["v3"] = m.group(1)
        op.compile("v3")
    return op


def _chunks(FC: int, tail_split: int):
    """Chunk (start, size, is_tail) list: big FC chunks, the last split into
    smaller pieces to shrink the end-of-kernel pipeline latency."""
    sizes = [(FC, False)] * (NPX // FC)
    if tail_split == 2:
        sizes = sizes[:-1] + [(FC // 2, True)] * 2
    elif tail_split == 3:
        sizes = sizes[:-1] + [(FC // 2, True)] + [(FC // 4, True)] * 2
    elif tail_split == 4:
        sizes = sizes[:-1] + [(FC // 4, True)] * 4
    out, pos = [], 0
    for s, tail in sizes:
        out.append((pos, s, tail))
        pos += s
    assert pos == NPX
    return out


def _build(scale_val: float, offset_val: float, reps: int = 1,
           FC: int = FC, work_bufs: int = 4, xq_bufs: int = 4,
           out_bufs: int = 3, ps_bufs: int = 4, out_dma: str = "scalar",
           in_dma: str = "sync", double_row: bool = True,
           tail_split: int = 2, ps_width: int = 1024,
           skip_out_dma: bool = False, out_phase: bool = True):
    op = _register_cliptrunc()
    nc = bacc.Bacc("TRN2", target_bir_lowering=False)
    x = nc.dram_tensor("x", [CIN, NPX], mybir.dt.float32, kind="ExternalInput")
    wt = nc.dram_tensor("wt", [128, 2, COUT], mybir.dt.float8e4,
                        kind="ExternalInput")
    out = nc.dram_tensor("out", [COUT, NPX], mybir.dt.int8,
                         kind="ExternalOutput")
    chunk_list = _chunks(FC, tail_split)

    with tile.TileContext(nc) as tc, \
         tc.tile_pool(name="singles", bufs=1) as singles, \
         tc.tile_pool(name="work", bufs=work_bufs) as work, \
         tc.tile_pool(name="xqp", bufs=xq_bufs) as xqp, \
         tc.tile_pool(name="outs", bufs=out_bufs) as outs, \
         tc.tile_pool(name="obig", bufs=2) as obigp, \
         tc.tile_pool(name="psum", bufs=ps_bufs, space="PSUM") as pspool:
        wtd = singles.tile([128, 2, COUT], mybir.dt.float8e4, tag="wtd")
        nc.scalar.dma_start(out=wtd[:, :, :], in_=wt[:, :, :])
        mg = singles.tile([128, 1], mybir.dt.float32, tag="mg")
        nc.vector.memset(mg[:, :], MAGIC)

        for rep in range(reps):
          obig = (obigp.tile([128, 2, NPX], mybir.dt.int8, tag="obig",
                             name="obig")
                  if out_phase else None)
          for c0, fc, is_tail in chunk_list:
            cols = slice(c0, c0 + fc)
            xq = xqp.tile([128, 2, FC], mybir.dt.float8e4, tag="xq")
            if in_dma == "fuse" and not is_tail:
                xr2 = work.tile([128, 2, FC], mybir.dt.float32, tag="xr2")
                nc.sync.dma_start(
                    out=xr2[:, :, :fc],
                    in_=x[:, cols].rearrange("(two p) n -> p two n", two=2))
                for ct in range(2):
                    nc.vector._custom_dve(op, out=xq[:, ct, :fc],
                                          in0=xr2[:, ct, :fc],
                                          in1=mg[:, :], s0=7.0, s1=-8.25,
                                          imm2=HALF)
            else:
                for ct in range(2):
                    xr = work.tile([128, FC], mybir.dt.float32, tag=f"xr{ct}")
                    eng_in = {"alt": ("sync", "scalar"),
                              "altpool": ("sync", "gpsimd"),
                              "fuse": ("sync", "sync")}.get(
                                  in_dma, (in_dma, in_dma))[ct]
                    getattr(nc, eng_in).dma_start(
                        out=xr[:, :fc], in_=x[ct * 128:(ct + 1) * 128, cols])
                    nc.vector._custom_dve(op, out=xq[:, ct, :fc],
                                          in0=xr[:, :fc],
                                          in1=mg[:, :], s0=7.0, s1=-8.25,
                                          imm2=HALF)
            for o in range(2):
                oi8 = (obig[:, o, :] if out_phase else
                       outs.tile([128, FC], mybir.dt.int8, tag=f"oi8{o}"))
                ob = c0 if out_phase else 0
                pw = min(fc, ps_width)
                for q in range(fc // pw):
                    ps = pspool.tile([128, ps_width], mybir.dt.float32,
                                     tag="ps")
                    for k in range(pw // 256):
                        nb = q * (pw // 256) + k
                        if double_row:
                            nc.tensor.matmul(
                                ps[:, k * 256:(k + 1) * 256],
                                wtd[:, :, o * 128:(o + 1) * 128],
                                xq[:, :, nb * 256:(nb + 1) * 256],
                                start=True, stop=True,
                                perf_mode=mybir.MatmulPerfMode.DoubleRow,
                            )
                        else:
                            for ct in range(2):
                                nc.tensor.matmul(
                                    ps[:, k * 256:(k + 1) * 256],
                                    wtd[:, ct, o * 128:(o + 1) * 128],
                                    xq[:, ct, nb * 256:(nb + 1) * 256],
                                    start=(ct == 0), stop=(ct == 1),
                                )
                    nc.scalar.activation(
                        out=oi8[:, ob + q * pw:ob + (q + 1) * pw],
                        in_=ps[:, :pw],
                        func=mybir.ActivationFunctionType.Copy,
                        scale=scale_val / 64.0, bias=offset_val,
                    )
                if out_phase:
                    continue  # drained in the per-rep output phase below
                if skip_out_dma and not (is_tail and o == 1 and
                                         c0 + fc == NPX):
                    continue  # timing probe only: drop output traffic
                eng = "scalar" if is_tail else out_dma
                getattr(nc, eng).dma_start(
                    out=out[o * 128:(o + 1) * 128, cols], in_=oi8[:, :fc])
          if out_phase:
            for o in range(2):
                getattr(nc, (out_dma, "scalar")[o]).dma_start(
                    out=out[o * 128:(o + 1) * 128, :], in_=obig[:, o, :])
    nc.finalize()
    return nc


_KERNEL_CACHE: dict = {}


def _prep_wt(weights: np.ndarray) -> np.ndarray:
    """Host-side DoubleRow weight layout: wt[p, i, oc] = W[oc, p + 128*i]."""
    dt_f8 = mybir.dt.np(mybir.dt.float8e4)
    wt = weights.astype(np.float32).T            # [Cin, Cout]
    wtd = wt.reshape(2, 128, COUT).transpose(1, 0, 2)
    return np.ascontiguousarray(wtd).astype(dt_f8)


def _prep_in_maps(x: np.ndarray, weights: np.ndarray) -> list:
    wtd = _prep_wt(weights)
    return [
        {"x": np.ascontiguousarray(x[b].reshape(CIN, NPX)), "wt": wtd}
        for b in range(B)
    ]


def kernel(x, weights, scale, offset):
    x = np.asarray(x)
    weights = np.asarray(weights)
    sv = float(np.asarray(scale))
    ov = float(np.asarray(offset))

    key = (sv, ov)
    if key not in _KERNEL_CACHE:
        _KERNEL_CACHE[key] = _build(sv, ov)
    nc = _KERNEL_CACHE[key]

    in_maps = _prep_in_maps(x, weights)
    res = run_bass_kernel_spmd(nc, in_maps, core_ids=list(range(B)),
                               trace=TRACE)
    _LAST_RESULTS[0] = res
    return np.stack([r["out"].reshape(COUT, H, W) for r in res.results])
